# revision 16
# baseline (speedup 1.0000x reference)
"""Multi-head causal attention (B=4, T=2048, D=1024, H=16) on 8 trn2 NeuronCores.

Sharding: core c handles batch b = c//2 and head-group g = c%2 (8 heads each).
Each core computes Q/K/V projections for its 8 heads, causal attention, and a
row-shard of the output projection; the host sums the two partial outputs per
batch (the "all-reduce") and adds the (b_v @ w_o + b_o) bias term.

Device algebra notes:
  - b_k drops out of softmax entirely (adds a per-query constant to scores).
  - b_v contributes exactly (b_v @ w_o) to every output row -> folded into the
    host-side bias along with b_o.
  - Scores are computed transposed ([k, q] layout) so that softmax sums ride
    free on the AV matmul (ones-column appended to V) and the AV output comes
    out as AV^T, which feeds the w_o matmul with no extra transposes.
  - Softmax denominators use reciprocal_approx_fast (~51 ULP): the exact
    iterative-divide reciprocal is ~6 cycles/elem on DVE and was a hidden
    critical-path hog; bf16 AV noise dwarfs the approx error.
  - Exp is issued once per PAIR of k-tiles ([128, 2, 2, 512] from one PSUM
    region): the ACT engine is the s3 bottleneck and pays ~290ns fixed cost
    per instruction, so halving the instruction count matters. For diagonal
    tile pairs the odd member's [q0-128, q0) columns get exp'd garbage —
    never read (AV and the mask-mul slice per-subtile).
  - Output partials are stored bf16 (halves store traffic); host upcasts.

DMA rule: hardware DMA descriptors encode at most ONE semaphore wait, so every
DMA destination here is written exactly once (no pool-slot reuse for DMA
targets); partition broadcasts are done with PE outer products, not DMA.
"""

import math
from contextlib import ExitStack

import numpy as np

import concourse.bass as bass
import concourse.mybir as mybir
import concourse.tile as tile
from concourse import bacc
from concourse.bass_utils import run_bass_kernel_spmd


FP = mybir.dt.float32
BF = mybir.dt.bfloat16

D_MODEL = 1024
N_HEADS = 16
B_FULL, T_FULL = 4, 2048
DK = 64                    # head dim
HPC = 8                    # heads per core
DH = HPC * DK              # 512 head-dims per core
N_CORES = 8


def build_bass(seq_len=T_FULL, causal=True, repeat=1, stages='123F',
               score_pack=True, pipe=3, pair_feeds=5):
    """Build the per-core Bass kernel (SPMD; same NEFF on all 8 cores).

    repeat > 1 wraps the whole kernel in a hardware loop — used only for
    benchmarking (amortizes host dispatch to time the kernel itself).
    """
    T = seq_len
    NT = T // 128             # t-tiles
    NCH = T // 512            # 512-wide t/q chunks
    ND = D_MODEL // 128       # d_model tiles (8)
    NM = DH // 128            # head-pair tiles (4)

    nc = bacc.Bacc("TRN2", target_bir_lowering=False, debug=False)
    # inputs arrive pre-cast to bf16 (host does the fp32->bf16 conversion)
    xbf_d = nc.dram_tensor("xbf", [T, D_MODEL], BF, kind="ExternalInput")
    wq_d = nc.dram_tensor("wq", [D_MODEL, DH], BF, kind="ExternalInput")
    wk_d = nc.dram_tensor("wk", [D_MODEL, DH], BF, kind="ExternalInput")
    wv_d = nc.dram_tensor("wv", [D_MODEL, DH], BF, kind="ExternalInput")
    wo_d = nc.dram_tensor("wo", [DH, D_MODEL], BF, kind="ExternalInput")
    bq_d = nc.dram_tensor("bq", [128, DH // 128], FP, kind="ExternalInput")
    out_d = nc.dram_tensor("out", [T, D_MODEL], BF, kind="ExternalOutput")

    with ExitStack() as ctx:
        tc = ctx.enter_context(tile.TileContext(nc))
        persist = ctx.enter_context(tc.tile_pool(name="persist", bufs=1))
        qt_pool = ctx.enter_context(tc.tile_pool(name="qt", bufs=3))
        xt_pool = ctx.enter_context(tc.tile_pool(name="xt", bufs=3))
        at_pool = ctx.enter_context(tc.tile_pool(name="atp", bufs=6))
        rec_pool = ctx.enter_context(tc.tile_pool(name="rec", bufs=2))
        avn_pool = ctx.enter_context(tc.tile_pool(name="avn", bufs=2))
        out_pool = ctx.enter_context(tc.tile_pool(name="outp", bufs=3))
        mm_ps = ctx.enter_context(tc.tile_pool(name="mmps", bufs=2, space="PSUM"))
        sc_ps = ctx.enter_context(tc.tile_pool(name="scps", bufs=2, space="PSUM"))
        av_ps = ctx.enter_context(tc.tile_pool(name="avps", bufs=1, space="PSUM"))

        def emit_kernel():
            # ---- weights + first x chunk first, so PE starts ASAP ----------
            def load_weight_bf(dram, n_tiles, cols, label):
                wsb = persist.tile([128, n_tiles, cols], BF, name=f"w_{label}",
                                   tag=f"w_{label}")
                nc.sync.dma_start(
                    out=wsb,
                    in_=dram.ap().rearrange("(j p) c -> p j c", p=128))
                return [wsb[:, j, :] for j in range(n_tiles)]

            def emit_s1(n):
                xT = [xt_pool.tile([128, 512], BF, name=f"xT{j}",
                                   tag=f"xT{j}") for j in range(ND)]
                for j in range(ND):
                    nc.sync.dma_start(out=xT[j],
                                      in_=xbf_d[n * 512:(n + 1) * 512,
                                                j * 128:(j + 1) * 128],
                                      transpose=True)
                return xT

            wk_bf = load_weight_bf(wk_d, ND, DH, "k")
            xT_cur = emit_s1(0)
            wq_bf = load_weight_bf(wq_d, ND, DH, "q")
            bq_sb = persist.tile([128, NM], FP, name="bq_sb", tag="bq_sb")
            nc.sync.dma_start(out=bq_sb, in_=bq_d[:, :])
            wv_bf = load_weight_bf(wv_d, ND, DH, "v")
            wo_bf = load_weight_bf(wo_d, NM, D_MODEL, "o")

            # ---- constants -------------------------------------------------
            ones_bf = persist.tile([1, DK], BF, name="ones_bf", tag="ones_bf")
            nc.gpsimd.memset(ones_bf, 1.0)

            masks = []
            if causal:
                for j in range(4):
                    m = persist.tile([128, 512], BF, name=f"mask{j}", tag=f"mask{j}")
                    nc.gpsimd.memset(m, 1.0)
                    # keep where (q - k - 128*j) >= 0, else 0
                    nc.gpsimd.affine_select(
                        out=m, in_=m, compare_op=mybir.AluOpType.is_ge,
                        fill=0.0, base=-128 * j, pattern=[[1, 512]],
                        channel_multiplier=-1)
                    masks.append(m)

            def bcast_mid(ap, nmid):
                return bass.AP(tensor=ap.tensor, offset=ap.offset,
                               ap=[list(ap.ap[0]), [0, nmid], list(ap.ap[1])])

            # ---- persistent per-core tensors ------------------------------
            # KT[mt]: [128, T] bf16, rows = head-dim (pair mt: heads 2mt,2mt+1)
            KT = [persist.tile([128, T], BF, name=f"KT{mt}", tag=f"KT{mt}")
                  for mt in range(NM)]
            # V natural layout with ones column: [128 t, 8 heads, 64+1]
            V_sb = [persist.tile([128, HPC, DK + 1], BF, name=f"V{i}", tag=f"V{i}")
                    for i in range(NT)]
            for i in range(NT):
                nc.gpsimd.memset(V_sb[i][:, :, DK], 1.0)
            # AV^T, write-once (DMA target for the h1 partition shift)
            AVT_all = [[persist.tile([128, 512], BF, name=f"AVT{n}_{p}",
                                     tag=f"AVT{n}_{p}") for p in range(NM)]
                       for n in range(NCH)]

            # ---- main streamed loop over 512-wide chunks -------------------
            # S2(n+1) and F(n-1) matmul groups are fed into S3(n)'s group
            # boundaries: the per-engine queues are in-order, so S3's
            # exp-latency bubbles can only be filled by positionally
            # interleaving independent work into its instruction stream.

            def make_s2_groups(n, xT):
                csl = bass.ts(n, 512)
                QT = [qt_pool.tile([128, 512], BF, name=f"QT{mt}",
                                   tag=f"QT{mt}") for mt in range(NM)]
                groups = []

                def qgroup(mt):
                    msl = bass.ts(mt, 128)
                    ps = mm_ps.tile([128, 512], FP, name="psq", tag="mm")
                    for j in range(ND):
                        nc.tensor.matmul(ps, wq_bf[j][:, msl], xT[j],
                                         start=(j == 0), stop=(j == ND - 1))
                    nc.vector.tensor_scalar_add(QT[mt], ps, bq_sb[:, mt:mt + 1])

                def kgroup(mt):
                    msl = bass.ts(mt, 128)
                    ps = mm_ps.tile([128, 512], FP, name="psk", tag="mm")
                    for j in range(ND):
                        nc.tensor.matmul(ps, wk_bf[j][:, msl], xT[j],
                                         start=(j == 0), stop=(j == ND - 1))
                    # fold the 1/sqrt(dk) score scale into K^T so exp needs
                    # no scale and score magnitudes stay ~N(0,1)
                    nc.vector.tensor_scalar_mul(KT[mt][:, csl], ps,
                                                1.0 / math.sqrt(DK))

                def vgroup(il):
                    i = 4 * n + il
                    ps = mm_ps.tile([128, 512], FP, name="psv", tag="mm")
                    for j in range(ND):
                        nc.tensor.matmul(ps, xT[j][:, il * 128:(il + 1) * 128],
                                         wv_bf[j], start=(j == 0),
                                         stop=(j == ND - 1))
                    nc.vector.tensor_copy(
                        V_sb[i][:, :, 0:DK],
                        ps.rearrange("p (h d) -> p h d", h=HPC))

                if '2' in stages:
                    if n == 0:
                        # chunk 0 runs un-interleaved at kernel start: order
                        # groups to match weight DMA arrival (wk, wq, wv)
                        for mt in range(NM):
                            groups.append(lambda mt=mt: kgroup(mt))
                        for mt in range(NM):
                            groups.append(lambda mt=mt: qgroup(mt))
                    else:
                        for mt in range(NM):
                            groups.append(lambda mt=mt: kgroup(mt))
                            groups.append(lambda mt=mt: qgroup(mt))
                    for il in range(4):
                        groups.append(lambda il=il: vgroup(il))
                return QT, groups

            def make_f_groups(n):
                AVT = AVT_all[n]
                groups = []

                osbs = {}

                def fgroup(il, cc):
                    i = 4 * n + il
                    isl = bass.ts(il, 128)
                    if il not in osbs:
                        osbs[il] = out_pool.tile([128, 2, 512], BF,
                                                 name="osb", tag="osb")
                    osb = osbs[il]
                    ps = mm_ps.tile([128, 512], FP, name="pso", tag="mm")
                    for dk in range(NM):
                        nc.tensor.matmul(
                            ps, AVT[dk][:, isl],
                            wo_bf[dk][:, cc * 512:(cc + 1) * 512],
                            start=(dk == 0), stop=(dk == NM - 1))
                    nc.vector.tensor_copy(osb[:, cc, :], ps)
                    nc.sync.dma_start(
                        out=out_d[i * 128:(i + 1) * 128,
                                  cc * 512:(cc + 1) * 512],
                        in_=osb[:, cc, :])

                if 'F' in stages:
                    for il in range(4):
                        for cc in range(2):
                            groups.append(lambda il=il, cc=cc: fgroup(il, cc))
                return groups

            def emit_s3(n, QT, feed):
                AVT = AVT_all[n]
                nkt = 4 * n + 4 if causal else NT
                PIPE = pipe
                pending_norm = [None]

                def emit_norm(p, av):
                    den_bf = rec_pool.tile([1, 2, 512], BF, name="den_bf",
                                           tag="den_bf")
                    nc.vector.tensor_copy(den_bf, av[DK:DK + 1, :, :])
                    rb_sb = rec_pool.tile([DK, 2, 512], FP, name="rb_sb",
                                          tag="rb_sb")
                    for hh in range(2):
                        rb = mm_ps.tile([DK, 512], FP, name=f"rb{hh}",
                                        tag="mm")
                        nc.tensor.matmul(rb, ones_bf, den_bf[:, hh, :],
                                         start=True, stop=True)
                        nc.vector.reciprocal_approx_fast(rb_sb[:, hh, :], rb)
                    nc.vector.tensor_mul(AVT[p][0:64, :], av[0:DK, 0, :],
                                         rb_sb[:, 0, :])
                    avn1 = avn_pool.tile([64, 512], BF, name="avn1",
                                         tag="avn1")
                    nc.vector.tensor_mul(avn1, av[0:DK, 1, :], rb_sb[:, 1, :])
                    nc.sync.dma_start(out=AVT[p][64:128, :], in_=avn1)

                npairs = NM if '3' in stages else 0
                for p in range(npairs):
                    av = av_ps.tile([DK + 1, 2, 512], FP, name="av", tag="av")

                    def emit_av(kt, at, q0):
                        for hh in range(2):
                            nc.tensor.matmul(
                                av[:, hh, q0:512],
                                V_sb[kt][:, 2 * p + hh, :],
                                at[:, hh, q0:512],
                                start=(kt == 0), stop=(kt == nkt - 1),
                                skip_group_check=True)

                    pend = []
                    for kt in range(nkt):
                        ksl = bass.ts(kt, 128)
                        # columns < j*128 of a diagonal tile are fully
                        # masked: skip in scores matmul / exp / mask / AV
                        j = kt - 4 * n if (causal and kt >= 4 * n) else 0
                        q0 = j * 128
                        ps_s = sc_ps.tile([128, 2, 512], FP, name="ps_s",
                                          tag="sc")
                        at = at_pool.tile([128, 2, 512], BF, name="at",
                                          tag="at")
                        for hh in range(2):
                            nc.tensor.matmul(
                                ps_s[:, hh, q0:512],
                                KT[p][hh * 64:(hh + 1) * 64, ksl],
                                QT[p][hh * 64:(hh + 1) * 64, q0:512],
                                start=True, stop=True,
                                tile_position=((hh * 64, 0) if score_pack
                                               else (0, 0)))
                        nc.scalar.activation(at[:, :, q0:512],
                                             ps_s[:, :, q0:512],
                                             mybir.ActivationFunctionType.Exp)
                        if causal and kt >= 4 * n:
                            # only the 128-wide diagonal block is partially
                            # masked; columns >= q0+128 are fully unmasked
                            q1 = q0 + 128
                            nc.vector.tensor_mul(
                                at[:, :, q0:q1], at[:, :, q0:q1],
                                bcast_mid(masks[j][:, q0:q1], 2))
                        if kt == 0 and pending_norm[0] is not None:
                            pending_norm[0]()
                            pending_norm[0] = None
                        pend.append((kt, at, q0))
                        if len(pend) > PIPE:
                            emit_av(*pend.pop(0))
                    for item in pend:
                        emit_av(*item)
                    pending_norm[0] = (lambda p=p, av=av:
                                       emit_norm(p, av))
                    # fill the pair-transition bubble with independent work
                    for _ in range(pair_feeds):
                        if feed:
                            feed.pop(0)()
                if pending_norm[0] is not None:
                    pending_norm[0]()
                    pending_norm[0] = None
                while feed:
                    feed.pop(0)()

            QT_cur, s2g = make_s2_groups(0, xT_cur)
            for g in s2g:
                g()
            f_prev = []
            for n in range(NCH):
                feed = []
                if n + 1 < NCH:
                    xT_next = emit_s1(n + 1)
                    QT_next, s2g_next = make_s2_groups(n + 1, xT_next)
                    feed += s2g_next
                else:
                    QT_next = None
                feed += f_prev
                if '3' in stages:
                    emit_s3(n, QT_cur, feed)
                else:
                    for g in feed:
                        g()
                f_prev = make_f_groups(n)
                QT_cur = QT_next
            for g in f_prev:
                g()

        if repeat > 1:
            with tc.For_i(0, repeat, 1):
                emit_kernel()
        else:
            emit_kernel()

    nc.compile()
    return nc


_NC_CACHE = {}


def _get_nc(seq_len, causal):
    key = (seq_len, causal)
    if key not in _NC_CACHE:
        _NC_CACHE[key] = build_bass(seq_len, causal)
    return _NC_CACHE[key]


def make_in_maps(x, w_q, b_q, w_k, w_v, w_o):
    """Per-core input dicts for the 8 cores (weights/x pre-cast to bf16)."""
    import ml_dtypes
    bf = ml_dtypes.bfloat16
    x_bf = np.ascontiguousarray(x).astype(bf)
    wq_bf = w_q.astype(bf)
    wk_bf = w_k.astype(bf)
    wv_bf = w_v.astype(bf)
    wo_bf = w_o.astype(bf)
    in_maps = []
    for c in range(N_CORES):
        b, g = divmod(c, 2)
        sl = slice(g * DH, (g + 1) * DH)
        in_maps.append({
            "xbf": x_bf[b],
            "wq": np.ascontiguousarray(wq_bf[:, sl]),
            "wk": np.ascontiguousarray(wk_bf[:, sl]),
            "wv": np.ascontiguousarray(wv_bf[:, sl]),
            "wo": np.ascontiguousarray(wo_bf[sl, :]),
            "bq": np.ascontiguousarray(
                b_q[sl].reshape(DH // 128, 128).T.astype(np.float32)),
        })
    return in_maps


def kernel(x, mask, w_q, b_q, w_k, b_k, w_v, b_v, w_o, b_o, _trace=False):
    x = np.asarray(x, dtype=np.float32)
    mask_np = np.asarray(mask).reshape(mask.shape[-2], mask.shape[-1])
    w_q, b_q = np.asarray(w_q, np.float32), np.asarray(b_q, np.float32)
    w_k = np.asarray(w_k, np.float32)
    w_v, b_v = np.asarray(w_v, np.float32), np.asarray(b_v, np.float32)
    w_o, b_o = np.asarray(w_o, np.float32), np.asarray(b_o, np.float32)

    T = x.shape[1]
    tril = np.tril(np.ones((T, T), dtype=mask_np.dtype))
    if np.array_equal(mask_np, tril):
        causal = True
    elif np.all(mask_np != 0):
        causal = False
    else:
        raise NotImplementedError("only causal or all-ones masks supported")

    nc = _get_nc(T, causal)
    in_maps = make_in_maps(x, w_q, b_q, w_k, w_v, w_o)
    res = run_bass_kernel_spmd(nc, in_maps, core_ids=list(range(N_CORES)),
                               trace=_trace)

    host_bias = (b_v @ w_o + b_o).astype(np.float32)
    out = np.empty((x.shape[0], T, D_MODEL), dtype=np.float32)
    for b in range(x.shape[0]):
        out[b] = np.asarray(res.results[2 * b]["out"], np.float32) \
            + np.asarray(res.results[2 * b + 1]["out"], np.float32) \
            + host_bias
    kernel._last_result = res
    return out


# revision 26
# speedup vs baseline: 1.0346x; 1.0346x over previous
"""Multi-head causal attention (B=4, T=2048, D=1024, H=16) on 8 trn2 NeuronCores.

Sharding: core c handles batch b = c//2 and head-group g = c%2 (8 heads each).
Each core computes Q/K/V projections for its 8 heads, causal attention, and a
row-shard of the output projection; the host sums the two partial outputs per
batch (the "all-reduce") and adds the (b_v @ w_o + b_o) bias term.

Device algebra notes:
  - b_k drops out of softmax entirely (adds a per-query constant to scores).
  - b_v contributes exactly (b_v @ w_o) to every output row -> folded into the
    host-side bias along with b_o.
  - Scores are computed transposed ([k, q] layout) so that softmax sums ride
    free on the AV matmul (ones-column appended to V) and the AV output comes
    out as AV^T, which feeds the w_o matmul with no extra transposes.
  - Softmax denominators use reciprocal_approx_fast (~51 ULP): the exact
    iterative-divide reciprocal is ~6 cycles/elem on DVE and was a hidden
    critical-path hog; bf16 AV noise dwarfs the approx error.
  - Exp is issued once per PAIR of k-tiles ([128, 2, 2, 512] from one PSUM
    region): the ACT engine is the s3 bottleneck and pays ~290ns fixed cost
    per instruction, so halving the instruction count matters. For diagonal
    tile pairs the odd member's [q0-128, q0) columns get exp'd garbage —
    never read (AV and the mask-mul slice per-subtile).
  - Output partials are stored bf16 (halves store traffic); host upcasts.

DMA rule: hardware DMA descriptors encode at most ONE semaphore wait, so every
DMA destination here is written exactly once (no pool-slot reuse for DMA
targets); partition broadcasts are done with PE outer products, not DMA.
"""

import math
from contextlib import ExitStack

import numpy as np

import concourse.bass as bass
import concourse.mybir as mybir
import concourse.tile as tile
from concourse import bacc
from concourse.bass_utils import run_bass_kernel_spmd


FP = mybir.dt.float32
BF = mybir.dt.bfloat16

D_MODEL = 1024
N_HEADS = 16
B_FULL, T_FULL = 4, 2048
DK = 64                    # head dim
HPC = 8                    # heads per core
DH = HPC * DK              # 512 head-dims per core
N_CORES = 8


def build_bass(seq_len=T_FULL, causal=True, repeat=1, stages='123F',
               score_pack=True, pipe=3, pair_feeds=5, s2_order0=True):
    """Build the per-core Bass kernel (SPMD; same NEFF on all 8 cores).

    repeat > 1 wraps the whole kernel in a hardware loop — used only for
    benchmarking (amortizes host dispatch to time the kernel itself).
    """
    T = seq_len
    NT = T // 128             # t-tiles
    NCH = T // 512            # 512-wide t/q chunks
    ND = D_MODEL // 128       # d_model tiles (8)
    NM = DH // 128            # head-pair tiles (4)

    nc = bacc.Bacc("TRN2", target_bir_lowering=False, debug=False)
    # inputs arrive pre-cast to bf16 AND pre-transposed (host does both):
    # xbf is x^T [D_MODEL, T], so the on-device x^T tile loads are plain
    # contiguous DMAs instead of xbar-transpose DMAs.
    xbf_d = nc.dram_tensor("xbf", [D_MODEL, T], BF, kind="ExternalInput")
    # weights arrive pre-arranged to the SBUF tile layout [p, j, c]
    # (host permutes), so each load is one contiguous-per-partition DMA
    wq_d = nc.dram_tensor("wq", [128, ND, DH], BF, kind="ExternalInput")
    wk_d = nc.dram_tensor("wk", [128, ND, DH], BF, kind="ExternalInput")
    wv_d = nc.dram_tensor("wv", [128, ND, DH], BF, kind="ExternalInput")
    wo_d = nc.dram_tensor("wo", [128, NM, D_MODEL], BF, kind="ExternalInput")
    bq_d = nc.dram_tensor("bq", [128, DH // 128], FP, kind="ExternalInput")
    out_d = nc.dram_tensor("out", [T, D_MODEL], BF, kind="ExternalOutput")

    with ExitStack() as ctx:
        tc = ctx.enter_context(tile.TileContext(nc))
        persist = ctx.enter_context(tc.tile_pool(name="persist", bufs=1))
        qt_pool = ctx.enter_context(tc.tile_pool(name="qt", bufs=3))
        xt_pool = ctx.enter_context(tc.tile_pool(name="xt", bufs=3))
        at_pool = ctx.enter_context(tc.tile_pool(name="atp", bufs=6))
        rec_pool = ctx.enter_context(tc.tile_pool(name="rec", bufs=2))
        avn_pool = ctx.enter_context(tc.tile_pool(name="avn", bufs=2))
        out_pool = ctx.enter_context(tc.tile_pool(name="outp", bufs=3))
        mm_ps = ctx.enter_context(tc.tile_pool(name="mmps", bufs=2, space="PSUM"))
        sc_ps = ctx.enter_context(tc.tile_pool(name="scps", bufs=2, space="PSUM"))
        av_ps = ctx.enter_context(tc.tile_pool(name="avps", bufs=1, space="PSUM"))

        def emit_kernel():
            # ---- weights + first x chunk first, so PE starts ASAP ----------
            def load_weight_bf(dram, n_tiles, cols, label):
                wsb = persist.tile([128, n_tiles, cols], BF, name=f"w_{label}",
                                   tag=f"w_{label}")
                nc.sync.dma_start(out=wsb, in_=dram.ap())
                return [wsb[:, j, :] for j in range(n_tiles)]

            def emit_s1(n):
                xT = [xt_pool.tile([128, 512], BF, name=f"xT{j}",
                                   tag=f"xT{j}") for j in range(ND)]
                for j in range(ND):
                    nc.sync.dma_start(out=xT[j],
                                      in_=xbf_d[j * 128:(j + 1) * 128,
                                                n * 512:(n + 1) * 512])
                return xT

            wk_bf = load_weight_bf(wk_d, ND, DH, "k")
            xT_cur = emit_s1(0)
            wq_bf = load_weight_bf(wq_d, ND, DH, "q")
            bq_sb = persist.tile([128, NM], FP, name="bq_sb", tag="bq_sb")
            nc.sync.dma_start(out=bq_sb, in_=bq_d[:, :])
            wv_bf = load_weight_bf(wv_d, ND, DH, "v")
            wo_bf = load_weight_bf(wo_d, NM, D_MODEL, "o")

            # ---- constants -------------------------------------------------
            ones_bf = persist.tile([1, DK], BF, name="ones_bf", tag="ones_bf")
            nc.gpsimd.memset(ones_bf, 1.0)

            masks = []
            if causal:
                for j in range(4):
                    m = persist.tile([128, 512], BF, name=f"mask{j}", tag=f"mask{j}")
                    nc.gpsimd.memset(m, 1.0)
                    # keep where (q - k - 128*j) >= 0, else 0
                    nc.gpsimd.affine_select(
                        out=m, in_=m, compare_op=mybir.AluOpType.is_ge,
                        fill=0.0, base=-128 * j, pattern=[[1, 512]],
                        channel_multiplier=-1)
                    masks.append(m)

            def bcast_mid(ap, nmid):
                return bass.AP(tensor=ap.tensor, offset=ap.offset,
                               ap=[list(ap.ap[0]), [0, nmid], list(ap.ap[1])])

            # ---- persistent per-core tensors ------------------------------
            # KT[mt]: [128, T] bf16, rows = head-dim (pair mt: heads 2mt,2mt+1)
            KT = [persist.tile([128, T], BF, name=f"KT{mt}", tag=f"KT{mt}")
                  for mt in range(NM)]
            # V natural layout with ones column: [128 t, 8 heads, 64+1]
            V_sb = [persist.tile([128, HPC, DK + 1], BF, name=f"V{i}", tag=f"V{i}")
                    for i in range(NT)]
            for i in range(NT):
                nc.gpsimd.memset(V_sb[i][:, :, DK], 1.0)
            # AV^T, write-once (DMA target for the h1 partition shift)
            AVT_all = [[persist.tile([128, 512], BF, name=f"AVT{n}_{p}",
                                     tag=f"AVT{n}_{p}") for p in range(NM)]
                       for n in range(NCH)]

            # ---- main streamed loop over 512-wide chunks -------------------
            # S2(n+1) and F(n-1) matmul groups are fed into S3(n)'s group
            # boundaries: the per-engine queues are in-order, so S3's
            # exp-latency bubbles can only be filled by positionally
            # interleaving independent work into its instruction stream.

            def make_s2_groups(n, xT):
                csl = bass.ts(n, 512)
                QT = [qt_pool.tile([128, 512], BF, name=f"QT{mt}",
                                   tag=f"QT{mt}") for mt in range(NM)]
                groups = []

                def qgroup(mt):
                    msl = bass.ts(mt, 128)
                    ps = mm_ps.tile([128, 512], FP, name="psq", tag="mm")
                    for j in range(ND):
                        nc.tensor.matmul(ps, wq_bf[j][:, msl], xT[j],
                                         start=(j == 0), stop=(j == ND - 1))
                    nc.vector.tensor_scalar_add(QT[mt], ps, bq_sb[:, mt:mt + 1])

                def kgroup(mt):
                    msl = bass.ts(mt, 128)
                    ps = mm_ps.tile([128, 512], FP, name="psk", tag="mm")
                    for j in range(ND):
                        nc.tensor.matmul(ps, wk_bf[j][:, msl], xT[j],
                                         start=(j == 0), stop=(j == ND - 1))
                    # fold the 1/sqrt(dk) score scale into K^T so exp needs
                    # no scale and score magnitudes stay ~N(0,1)
                    nc.vector.tensor_scalar_mul(KT[mt][:, csl], ps,
                                                1.0 / math.sqrt(DK))

                def vgroup(il):
                    i = 4 * n + il
                    ps = mm_ps.tile([128, 512], FP, name="psv", tag="mm")
                    for j in range(ND):
                        nc.tensor.matmul(ps, xT[j][:, il * 128:(il + 1) * 128],
                                         wv_bf[j], start=(j == 0),
                                         stop=(j == ND - 1))
                    nc.vector.tensor_copy(
                        V_sb[i][:, :, 0:DK],
                        ps.rearrange("p (h d) -> p h d", h=HPC))

                if '2' in stages:
                    if n == 0 and s2_order0:
                        # chunk 0 runs un-interleaved at kernel start: order
                        # groups to match weight DMA arrival (wk, wq, wv)
                        for mt in range(NM):
                            groups.append(lambda mt=mt: kgroup(mt))
                        for mt in range(NM):
                            groups.append(lambda mt=mt: qgroup(mt))
                    else:
                        for mt in range(NM):
                            groups.append(lambda mt=mt: kgroup(mt))
                            groups.append(lambda mt=mt: qgroup(mt))
                    for il in range(4):
                        groups.append(lambda il=il: vgroup(il))
                return QT, groups

            def make_f_groups(n):
                AVT = AVT_all[n]
                groups = []

                osbs = {}

                def fgroup(il, cc):
                    i = 4 * n + il
                    isl = bass.ts(il, 128)
                    if il not in osbs:
                        osbs[il] = out_pool.tile([128, 2, 512], BF,
                                                 name="osb", tag="osb")
                    osb = osbs[il]
                    ps = mm_ps.tile([128, 512], FP, name="pso", tag="mm")
                    for dk in range(NM):
                        nc.tensor.matmul(
                            ps, AVT[dk][:, isl],
                            wo_bf[dk][:, cc * 512:(cc + 1) * 512],
                            start=(dk == 0), stop=(dk == NM - 1))
                    nc.vector.tensor_copy(osb[:, cc, :], ps)
                    nc.sync.dma_start(
                        out=out_d[i * 128:(i + 1) * 128,
                                  cc * 512:(cc + 1) * 512],
                        in_=osb[:, cc, :])

                if 'F' in stages:
                    for il in range(4):
                        for cc in range(2):
                            groups.append(lambda il=il, cc=cc: fgroup(il, cc))
                return groups

            def emit_s3(n, QT, feed):
                AVT = AVT_all[n]
                nkt = 4 * n + 4 if causal else NT
                PIPE = pipe
                pending_norm = [None]

                def emit_norm(p, av):
                    den_bf = rec_pool.tile([1, 2, 512], BF, name="den_bf",
                                           tag="den_bf")
                    nc.vector.tensor_copy(den_bf, av[DK:DK + 1, :, :])
                    rb_sb = rec_pool.tile([DK, 2, 512], FP, name="rb_sb",
                                          tag="rb_sb")
                    for hh in range(2):
                        rb = mm_ps.tile([DK, 512], FP, name=f"rb{hh}",
                                        tag="mm")
                        nc.tensor.matmul(rb, ones_bf, den_bf[:, hh, :],
                                         start=True, stop=True)
                        # ~6 cpe exact reciprocal is a hidden DVE hog; the
                        # ~51-ULP approx dwarfs bf16 AV noise
                        nc.vector.reciprocal_approx_fast(rb_sb[:, hh, :], rb)
                    nc.vector.tensor_mul(AVT[p][0:64, :], av[0:DK, 0, :],
                                         rb_sb[:, 0, :])
                    avn1 = avn_pool.tile([64, 512], BF, name="avn1",
                                         tag="avn1")
                    nc.vector.tensor_mul(avn1, av[0:DK, 1, :], rb_sb[:, 1, :])
                    nc.sync.dma_start(out=AVT[p][64:128, :], in_=avn1)

                npairs = NM if '3' in stages else 0
                for p in range(npairs):
                    av = av_ps.tile([DK + 1, 2, 512], FP, name="av", tag="av")

                    def emit_av(kt, at, q0):
                        for hh in range(2):
                            nc.tensor.matmul(
                                av[:, hh, q0:512],
                                V_sb[kt][:, 2 * p + hh, :],
                                at[:, hh, q0:512],
                                start=(kt == 0), stop=(kt == nkt - 1),
                                skip_group_check=True)

                    pend = []
                    for kt in range(nkt):
                        ksl = bass.ts(kt, 128)
                        # columns < j*128 of a diagonal tile are fully
                        # masked: skip in scores matmul / exp / mask / AV
                        j = kt - 4 * n if (causal and kt >= 4 * n) else 0
                        q0 = j * 128
                        ps_s = sc_ps.tile([128, 2, 512], FP, name="ps_s",
                                          tag="sc")
                        at = at_pool.tile([128, 2, 512], BF, name="at",
                                          tag="at")
                        for hh in range(2):
                            nc.tensor.matmul(
                                ps_s[:, hh, q0:512],
                                KT[p][hh * 64:(hh + 1) * 64, ksl],
                                QT[p][hh * 64:(hh + 1) * 64, q0:512],
                                start=True, stop=True,
                                tile_position=((hh * 64, 0) if score_pack
                                               else (0, 0)))
                        nc.scalar.activation(at[:, :, q0:512],
                                             ps_s[:, :, q0:512],
                                             mybir.ActivationFunctionType.Exp)
                        if causal and kt >= 4 * n:
                            # only the 128-wide diagonal block is partially
                            # masked; columns >= q0+128 are fully unmasked
                            q1 = q0 + 128
                            nc.vector.tensor_mul(
                                at[:, :, q0:q1], at[:, :, q0:q1],
                                bcast_mid(masks[j][:, q0:q1], 2))
                        if kt == 0 and pending_norm[0] is not None:
                            pending_norm[0]()
                            pending_norm[0] = None
                        pend.append((kt, at, q0))
                        if len(pend) > PIPE:
                            emit_av(*pend.pop(0))
                    for item in pend:
                        emit_av(*item)
                    pending_norm[0] = (lambda p=p, av=av:
                                       emit_norm(p, av))
                    # fill the pair-transition bubble with independent work
                    for _ in range(pair_feeds):
                        if feed:
                            feed.pop(0)()
                if pending_norm[0] is not None:
                    pending_norm[0]()
                    pending_norm[0] = None
                while feed:
                    feed.pop(0)()

            QT_cur, s2g = make_s2_groups(0, xT_cur)
            for g in s2g:
                g()
            f_prev = []
            for n in range(NCH):
                feed = []
                if n + 1 < NCH:
                    xT_next = emit_s1(n + 1)
                    QT_next, s2g_next = make_s2_groups(n + 1, xT_next)
                    feed += s2g_next
                else:
                    QT_next = None
                feed += f_prev
                if '3' in stages:
                    emit_s3(n, QT_cur, feed)
                else:
                    for g in feed:
                        g()
                f_prev = make_f_groups(n)
                QT_cur = QT_next
            for g in f_prev:
                g()

        if repeat > 1:
            with tc.For_i(0, repeat, 1):
                emit_kernel()
        else:
            emit_kernel()

    nc.compile()
    return nc


_NC_CACHE = {}


def _get_nc(seq_len, causal):
    key = (seq_len, causal)
    if key not in _NC_CACHE:
        _NC_CACHE[key] = build_bass(seq_len, causal)
    return _NC_CACHE[key]


def make_in_maps(x, w_q, b_q, w_k, w_v, w_o):
    """Per-core input dicts for the 8 cores (weights/x pre-cast to bf16)."""
    import ml_dtypes
    bf = ml_dtypes.bfloat16
    # pre-transpose per batch: device expects x^T [D_MODEL, T]
    x_bf = np.ascontiguousarray(np.asarray(x).transpose(0, 2, 1)).astype(bf)
    wq_bf = w_q.astype(bf)
    wk_bf = w_k.astype(bf)
    wv_bf = w_v.astype(bf)
    wo_bf = w_o.astype(bf)
    def sbuf_layout(w):  # [rows, cols] -> [128, rows//128, cols], p-major
        r, c = w.shape
        return np.ascontiguousarray(
            w.reshape(r // 128, 128, c).transpose(1, 0, 2))

    in_maps = []
    for c in range(N_CORES):
        b, g = divmod(c, 2)
        sl = slice(g * DH, (g + 1) * DH)
        in_maps.append({
            "xbf": x_bf[b],
            "wq": sbuf_layout(wq_bf[:, sl]),
            "wk": sbuf_layout(wk_bf[:, sl]),
            "wv": sbuf_layout(wv_bf[:, sl]),
            "wo": sbuf_layout(np.ascontiguousarray(wo_bf[sl, :])),
            "bq": np.ascontiguousarray(
                b_q[sl].reshape(DH // 128, 128).T.astype(np.float32)),
        })
    return in_maps


def kernel(x, mask, w_q, b_q, w_k, b_k, w_v, b_v, w_o, b_o, _trace=False):
    x = np.asarray(x, dtype=np.float32)
    mask_np = np.asarray(mask).reshape(mask.shape[-2], mask.shape[-1])
    w_q, b_q = np.asarray(w_q, np.float32), np.asarray(b_q, np.float32)
    w_k = np.asarray(w_k, np.float32)
    w_v, b_v = np.asarray(w_v, np.float32), np.asarray(b_v, np.float32)
    w_o, b_o = np.asarray(w_o, np.float32), np.asarray(b_o, np.float32)

    T = x.shape[1]
    tril = np.tril(np.ones((T, T), dtype=mask_np.dtype))
    if np.array_equal(mask_np, tril):
        causal = True
    elif np.all(mask_np != 0):
        causal = False
    else:
        raise NotImplementedError("only causal or all-ones masks supported")

    nc = _get_nc(T, causal)
    in_maps = make_in_maps(x, w_q, b_q, w_k, w_v, w_o)
    res = run_bass_kernel_spmd(nc, in_maps, core_ids=list(range(N_CORES)),
                               trace=_trace)

    host_bias = (b_v @ w_o + b_o).astype(np.float32)
    out = np.empty((x.shape[0], T, D_MODEL), dtype=np.float32)
    for b in range(x.shape[0]):
        out[b] = np.asarray(res.results[2 * b]["out"], np.float32) \
            + np.asarray(res.results[2 * b + 1]["out"], np.float32) \
            + host_bias
    kernel._last_result = res
    return out


# revision 30
# speedup vs baseline: 1.0763x; 1.0403x over previous
"""Multi-head causal attention (B=4, T=2048, D=1024, H=16) on 8 trn2 NeuronCores.

Sharding: core c handles batch b = c//2 and head-group g = c%2 (8 heads each).
Each core computes Q/K/V projections for its 8 heads, causal attention, and a
row-shard of the output projection; the host sums the two partial outputs per
batch (the "all-reduce") and adds the (b_v @ w_o + b_o) bias term.

Device algebra notes:
  - b_k drops out of softmax entirely (adds a per-query constant to scores).
  - b_v contributes exactly (b_v @ w_o) to every output row -> folded into the
    host-side bias along with b_o.
  - Scores are computed transposed ([k, q] layout) so that softmax sums ride
    free on the AV matmul (ones-column appended to V) and the AV output comes
    out as AV^T, which feeds the w_o matmul with no extra transposes.
  - Softmax denominators use reciprocal_approx_fast (~51 ULP): the exact
    iterative-divide reciprocal is ~6 cycles/elem on DVE and was a hidden
    critical-path hog; bf16 AV noise dwarfs the approx error.
  - Exp is issued once per PAIR of k-tiles ([128, 2, 2, 512] from one PSUM
    region): the ACT engine is the s3 bottleneck and pays ~290ns fixed cost
    per instruction, so halving the instruction count matters. For diagonal
    tile pairs the odd member's [q0-128, q0) columns get exp'd garbage —
    never read (AV and the mask-mul slice per-subtile).
  - Output partials are stored bf16 (halves store traffic); host upcasts.

DMA rule: hardware DMA descriptors encode at most ONE semaphore wait, so every
DMA destination here is written exactly once (no pool-slot reuse for DMA
targets); partition broadcasts are done with PE outer products, not DMA.
"""

import math
from contextlib import ExitStack

import numpy as np

import concourse.bass as bass
import concourse.mybir as mybir
import concourse.tile as tile
from concourse import bacc
from concourse.bass_utils import run_bass_kernel_spmd


FP = mybir.dt.float32
BF = mybir.dt.bfloat16

D_MODEL = 1024
N_HEADS = 16
B_FULL, T_FULL = 4, 2048
DK = 64                    # head dim
HPC = 8                    # heads per core
DH = HPC * DK              # 512 head-dims per core
N_CORES = 8


def build_bass(seq_len=T_FULL, causal=True, repeat=1, stages='123F',
               score_pack=True, pipe=3, pair_feeds=3, s2_order0=True,
               warmup=16):
    """Build the per-core Bass kernel (SPMD; same NEFF on all 8 cores).

    repeat > 1 wraps the whole kernel in a hardware loop — used only for
    benchmarking (amortizes host dispatch to time the kernel itself).
    """
    T = seq_len
    NT = T // 128             # t-tiles
    NCH = T // 512            # 512-wide t/q chunks
    ND = D_MODEL // 128       # d_model tiles (8)
    NM = DH // 128            # head-pair tiles (4)

    nc = bacc.Bacc("TRN2", target_bir_lowering=False, debug=False)
    # inputs arrive pre-cast to bf16 AND pre-transposed (host does both):
    # xbf is x^T [D_MODEL, T], so the on-device x^T tile loads are plain
    # contiguous DMAs instead of xbar-transpose DMAs.
    xbf_d = nc.dram_tensor("xbf", [D_MODEL, T], BF, kind="ExternalInput")
    # weights arrive pre-arranged to the SBUF tile layout [p, j, c]
    # (host permutes), so each load is one contiguous-per-partition DMA
    wq_d = nc.dram_tensor("wq", [128, ND, DH], BF, kind="ExternalInput")
    wk_d = nc.dram_tensor("wk", [128, ND, DH], BF, kind="ExternalInput")
    wv_d = nc.dram_tensor("wv", [128, ND, DH], BF, kind="ExternalInput")
    wo_d = nc.dram_tensor("wo", [128, NM, D_MODEL], BF, kind="ExternalInput")
    bq_d = nc.dram_tensor("bq", [128, DH // 128], FP, kind="ExternalInput")
    out_d = nc.dram_tensor("out", [T, D_MODEL], BF, kind="ExternalOutput")

    with ExitStack() as ctx:
        tc = ctx.enter_context(tile.TileContext(nc))
        persist = ctx.enter_context(tc.tile_pool(name="persist", bufs=1))
        qt_pool = ctx.enter_context(tc.tile_pool(name="qt", bufs=3))
        xt_pool = ctx.enter_context(tc.tile_pool(name="xt", bufs=3))
        at_pool = ctx.enter_context(tc.tile_pool(name="atp", bufs=6))
        rec_pool = ctx.enter_context(tc.tile_pool(name="rec", bufs=2))
        avn_pool = ctx.enter_context(tc.tile_pool(name="avn", bufs=2))
        out_pool = ctx.enter_context(tc.tile_pool(name="outp", bufs=3))
        mm_ps = ctx.enter_context(tc.tile_pool(name="mmps", bufs=2, space="PSUM"))
        sc_ps = ctx.enter_context(tc.tile_pool(name="scps", bufs=2, space="PSUM"))
        av_ps = ctx.enter_context(tc.tile_pool(name="avps", bufs=1, space="PSUM"))

        def emit_kernel():
            # ---- weights + first x chunk first, so PE starts ASAP ----------
            def load_weight_bf(dram, n_tiles, cols, label):
                wsb = persist.tile([128, n_tiles, cols], BF, name=f"w_{label}",
                                   tag=f"w_{label}")
                nc.sync.dma_start(out=wsb, in_=dram.ap())
                return [wsb[:, j, :] for j in range(n_tiles)]

            def emit_s1(n):
                xT = [xt_pool.tile([128, 512], BF, name=f"xT{j}",
                                   tag=f"xT{j}") for j in range(ND)]
                for j in range(ND):
                    nc.sync.dma_start(out=xT[j],
                                      in_=xbf_d[j * 128:(j + 1) * 128,
                                                n * 512:(n + 1) * 512])
                return xT

            wk_bf = load_weight_bf(wk_d, ND, DH, "k")
            xT_cur = emit_s1(0)
            wq_bf = load_weight_bf(wq_d, ND, DH, "q")
            bq_sb = persist.tile([128, NM], FP, name="bq_sb", tag="bq_sb")
            nc.sync.dma_start(out=bq_sb, in_=bq_d[:, :])
            wv_bf = load_weight_bf(wv_d, ND, DH, "v")
            wo_bf = load_weight_bf(wo_d, NM, D_MODEL, "o")

            # ---- constants -------------------------------------------------
            ones_bf = persist.tile([1, DK], BF, name="ones_bf", tag="ones_bf")
            nc.gpsimd.memset(ones_bf, 1.0)

            # optional HAM warm-up: dummy matmuls run while the first DMAs
            # land, keeping the PE activity window busy so real matmuls
            # start un-throttled (PE clock gate needs ~3.4us of activity)
            if warmup:
                warm_sb = persist.tile([1, 512], BF, name="warm_sb",
                                       tag="warm_sb")
                nc.gpsimd.memset(warm_sb, 1.0)
                for w in range(warmup):
                    wps = mm_ps.tile([DK, 512], FP, name="warm", tag="mm")
                    nc.tensor.matmul(wps, ones_bf, warm_sb,
                                     start=True, stop=True)

            masks = []
            if causal:
                for j in range(4):
                    m = persist.tile([128, 512], BF, name=f"mask{j}", tag=f"mask{j}")
                    nc.gpsimd.memset(m, 1.0)
                    # keep where (q - k - 128*j) >= 0, else 0
                    nc.gpsimd.affine_select(
                        out=m, in_=m, compare_op=mybir.AluOpType.is_ge,
                        fill=0.0, base=-128 * j, pattern=[[1, 512]],
                        channel_multiplier=-1)
                    masks.append(m)

            def bcast_mid(ap, nmid):
                return bass.AP(tensor=ap.tensor, offset=ap.offset,
                               ap=[list(ap.ap[0]), [0, nmid], list(ap.ap[1])])

            # ---- persistent per-core tensors ------------------------------
            # KT[mt]: [128, T] bf16, rows = head-dim (pair mt: heads 2mt,2mt+1)
            KT = [persist.tile([128, T], BF, name=f"KT{mt}", tag=f"KT{mt}")
                  for mt in range(NM)]
            # V natural layout with ones column: [128 t, 8 heads, 64+1]
            V_sb = [persist.tile([128, HPC, DK + 1], BF, name=f"V{i}", tag=f"V{i}")
                    for i in range(NT)]
            for i in range(NT):
                nc.gpsimd.memset(V_sb[i][:, :, DK], 1.0)
            # AV^T, write-once (DMA target for the h1 partition shift)
            AVT_all = [[persist.tile([128, 512], BF, name=f"AVT{n}_{p}",
                                     tag=f"AVT{n}_{p}") for p in range(NM)]
                       for n in range(NCH)]

            # ---- main streamed loop over 512-wide chunks -------------------
            # S2(n+1) and F(n-1) matmul groups are fed into S3(n)'s group
            # boundaries: the per-engine queues are in-order, so S3's
            # exp-latency bubbles can only be filled by positionally
            # interleaving independent work into its instruction stream.

            def make_s2_groups(n, xT):
                csl = bass.ts(n, 512)
                QT = [qt_pool.tile([128, 512], BF, name=f"QT{mt}",
                                   tag=f"QT{mt}") for mt in range(NM)]
                groups = []

                def qgroup(mt):
                    msl = bass.ts(mt, 128)
                    ps = mm_ps.tile([128, 512], FP, name="psq", tag="mm")
                    for j in range(ND):
                        nc.tensor.matmul(ps, wq_bf[j][:, msl], xT[j],
                                         start=(j == 0), stop=(j == ND - 1))
                    nc.vector.tensor_scalar_add(QT[mt], ps, bq_sb[:, mt:mt + 1])

                def kgroup(mt):
                    msl = bass.ts(mt, 128)
                    ps = mm_ps.tile([128, 512], FP, name="psk", tag="mm")
                    for j in range(ND):
                        nc.tensor.matmul(ps, wk_bf[j][:, msl], xT[j],
                                         start=(j == 0), stop=(j == ND - 1))
                    # fold the 1/sqrt(dk) score scale into K^T so exp needs
                    # no scale and score magnitudes stay ~N(0,1)
                    nc.vector.tensor_scalar_mul(KT[mt][:, csl], ps,
                                                1.0 / math.sqrt(DK))

                def vgroup(il):
                    i = 4 * n + il
                    ps = mm_ps.tile([128, 512], FP, name="psv", tag="mm")
                    for j in range(ND):
                        nc.tensor.matmul(ps, xT[j][:, il * 128:(il + 1) * 128],
                                         wv_bf[j], start=(j == 0),
                                         stop=(j == ND - 1))
                    nc.vector.tensor_copy(
                        V_sb[i][:, :, 0:DK],
                        ps.rearrange("p (h d) -> p h d", h=HPC))

                if '2' in stages:
                    if n == 0 and s2_order0:
                        # chunk 0 runs un-interleaved at kernel start: order
                        # groups to match weight DMA arrival (wk, wq, wv)
                        for mt in range(NM):
                            groups.append(lambda mt=mt: kgroup(mt))
                        for mt in range(NM):
                            groups.append(lambda mt=mt: qgroup(mt))
                    else:
                        for mt in range(NM):
                            groups.append(lambda mt=mt: kgroup(mt))
                            groups.append(lambda mt=mt: qgroup(mt))
                    for il in range(4):
                        groups.append(lambda il=il: vgroup(il))
                return QT, groups

            def make_f_groups(n):
                AVT = AVT_all[n]
                groups = []

                osbs = {}

                def fgroup(il, cc):
                    i = 4 * n + il
                    isl = bass.ts(il, 128)
                    if il not in osbs:
                        osbs[il] = out_pool.tile([128, 2, 512], BF,
                                                 name="osb", tag="osb")
                    osb = osbs[il]
                    ps = mm_ps.tile([128, 512], FP, name="pso", tag="mm")
                    for dk in range(NM):
                        nc.tensor.matmul(
                            ps, AVT[dk][:, isl],
                            wo_bf[dk][:, cc * 512:(cc + 1) * 512],
                            start=(dk == 0), stop=(dk == NM - 1))
                    nc.vector.tensor_copy(osb[:, cc, :], ps)
                    nc.sync.dma_start(
                        out=out_d[i * 128:(i + 1) * 128,
                                  cc * 512:(cc + 1) * 512],
                        in_=osb[:, cc, :])

                if 'F' in stages:
                    for il in range(4):
                        for cc in range(2):
                            groups.append(lambda il=il, cc=cc: fgroup(il, cc))
                return groups

            def emit_s3(n, QT, feed):
                AVT = AVT_all[n]
                nkt = 4 * n + 4 if causal else NT
                PIPE = pipe
                pending_norm = [None]

                def emit_norm(p, av):
                    den_bf = rec_pool.tile([1, 2, 512], BF, name="den_bf",
                                           tag="den_bf")
                    nc.vector.tensor_copy(den_bf, av[DK:DK + 1, :, :])
                    rb_sb = rec_pool.tile([DK, 2, 512], FP, name="rb_sb",
                                          tag="rb_sb")
                    for hh in range(2):
                        rb = mm_ps.tile([DK, 512], FP, name=f"rb{hh}",
                                        tag="mm")
                        nc.tensor.matmul(rb, ones_bf, den_bf[:, hh, :],
                                         start=True, stop=True)
                        # ~6 cpe exact reciprocal is a hidden DVE hog; the
                        # ~51-ULP approx dwarfs bf16 AV noise
                        nc.vector.reciprocal_approx_fast(rb_sb[:, hh, :], rb)
                    nc.vector.tensor_mul(AVT[p][0:64, :], av[0:DK, 0, :],
                                         rb_sb[:, 0, :])
                    avn1 = avn_pool.tile([64, 512], BF, name="avn1",
                                         tag="avn1")
                    nc.vector.tensor_mul(avn1, av[0:DK, 1, :], rb_sb[:, 1, :])
                    nc.sync.dma_start(out=AVT[p][64:128, :], in_=avn1)

                npairs = NM if '3' in stages else 0
                for p in range(npairs):
                    av = av_ps.tile([DK + 1, 2, 512], FP, name="av", tag="av")

                    def emit_av(kt, at, q0):
                        for hh in range(2):
                            nc.tensor.matmul(
                                av[:, hh, q0:512],
                                V_sb[kt][:, 2 * p + hh, :],
                                at[:, hh, q0:512],
                                start=(kt == 0), stop=(kt == nkt - 1),
                                skip_group_check=True)

                    pend = []
                    for kt in range(nkt):
                        ksl = bass.ts(kt, 128)
                        # columns < j*128 of a diagonal tile are fully
                        # masked: skip in scores matmul / exp / mask / AV
                        j = kt - 4 * n if (causal and kt >= 4 * n) else 0
                        q0 = j * 128
                        ps_s = sc_ps.tile([128, 2, 512], FP, name="ps_s",
                                          tag="sc")
                        at = at_pool.tile([128, 2, 512], BF, name="at",
                                          tag="at")
                        for hh in range(2):
                            nc.tensor.matmul(
                                ps_s[:, hh, q0:512],
                                KT[p][hh * 64:(hh + 1) * 64, ksl],
                                QT[p][hh * 64:(hh + 1) * 64, q0:512],
                                start=True, stop=True,
                                tile_position=((hh * 64, 0) if score_pack
                                               else (0, 0)))
                        nc.scalar.activation(at[:, :, q0:512],
                                             ps_s[:, :, q0:512],
                                             mybir.ActivationFunctionType.Exp)
                        if causal and kt >= 4 * n:
                            # only the 128-wide diagonal block is partially
                            # masked; columns >= q0+128 are fully unmasked
                            q1 = q0 + 128
                            nc.vector.tensor_mul(
                                at[:, :, q0:q1], at[:, :, q0:q1],
                                bcast_mid(masks[j][:, q0:q1], 2))
                        if kt == 0 and pending_norm[0] is not None:
                            pending_norm[0]()
                            pending_norm[0] = None
                        pend.append((kt, at, q0))
                        if len(pend) > PIPE:
                            emit_av(*pend.pop(0))
                    for item in pend:
                        emit_av(*item)
                    pending_norm[0] = (lambda p=p, av=av:
                                       emit_norm(p, av))
                    # fill the pair-transition bubble with independent work
                    for _ in range(pair_feeds):
                        if feed:
                            feed.pop(0)()
                if pending_norm[0] is not None:
                    pending_norm[0]()
                    pending_norm[0] = None
                while feed:
                    feed.pop(0)()

            QT_cur, s2g = make_s2_groups(0, xT_cur)
            for g in s2g:
                g()
            f_prev = []
            for n in range(NCH):
                feed = []
                if n + 1 < NCH:
                    xT_next = emit_s1(n + 1)
                    QT_next, s2g_next = make_s2_groups(n + 1, xT_next)
                    feed += s2g_next
                else:
                    QT_next = None
                feed += f_prev
                if '3' in stages:
                    emit_s3(n, QT_cur, feed)
                else:
                    for g in feed:
                        g()
                f_prev = make_f_groups(n)
                QT_cur = QT_next
            for g in f_prev:
                g()

        if repeat > 1:
            with tc.For_i(0, repeat, 1):
                emit_kernel()
        else:
            emit_kernel()

    nc.compile()
    return nc


_NC_CACHE = {}


def _get_nc(seq_len, causal):
    key = (seq_len, causal)
    if key not in _NC_CACHE:
        _NC_CACHE[key] = build_bass(seq_len, causal)
    return _NC_CACHE[key]


def make_in_maps(x, w_q, b_q, w_k, w_v, w_o):
    """Per-core input dicts for the 8 cores (weights/x pre-cast to bf16)."""
    import ml_dtypes
    bf = ml_dtypes.bfloat16
    # pre-transpose per batch: device expects x^T [D_MODEL, T]
    x_bf = np.ascontiguousarray(np.asarray(x).transpose(0, 2, 1)).astype(bf)
    wq_bf = w_q.astype(bf)
    wk_bf = w_k.astype(bf)
    wv_bf = w_v.astype(bf)
    wo_bf = w_o.astype(bf)
    def sbuf_layout(w):  # [rows, cols] -> [128, rows//128, cols], p-major
        r, c = w.shape
        return np.ascontiguousarray(
            w.reshape(r // 128, 128, c).transpose(1, 0, 2))

    in_maps = []
    for c in range(N_CORES):
        b, g = divmod(c, 2)
        sl = slice(g * DH, (g + 1) * DH)
        in_maps.append({
            "xbf": x_bf[b],
            "wq": sbuf_layout(wq_bf[:, sl]),
            "wk": sbuf_layout(wk_bf[:, sl]),
            "wv": sbuf_layout(wv_bf[:, sl]),
            "wo": sbuf_layout(np.ascontiguousarray(wo_bf[sl, :])),
            "bq": np.ascontiguousarray(
                b_q[sl].reshape(DH // 128, 128).T.astype(np.float32)),
        })
    return in_maps


def kernel(x, mask, w_q, b_q, w_k, b_k, w_v, b_v, w_o, b_o, _trace=False):
    x = np.asarray(x, dtype=np.float32)
    mask_np = np.asarray(mask).reshape(mask.shape[-2], mask.shape[-1])
    w_q, b_q = np.asarray(w_q, np.float32), np.asarray(b_q, np.float32)
    w_k = np.asarray(w_k, np.float32)
    w_v, b_v = np.asarray(w_v, np.float32), np.asarray(b_v, np.float32)
    w_o, b_o = np.asarray(w_o, np.float32), np.asarray(b_o, np.float32)

    T = x.shape[1]
    tril = np.tril(np.ones((T, T), dtype=mask_np.dtype))
    if np.array_equal(mask_np, tril):
        causal = True
    elif np.all(mask_np != 0):
        causal = False
    else:
        raise NotImplementedError("only causal or all-ones masks supported")

    nc = _get_nc(T, causal)
    in_maps = make_in_maps(x, w_q, b_q, w_k, w_v, w_o)
    res = run_bass_kernel_spmd(nc, in_maps, core_ids=list(range(N_CORES)),
                               trace=_trace)

    host_bias = (b_v @ w_o + b_o).astype(np.float32)
    out = np.empty((x.shape[0], T, D_MODEL), dtype=np.float32)
    for b in range(x.shape[0]):
        out[b] = np.asarray(res.results[2 * b]["out"], np.float32) \
            + np.asarray(res.results[2 * b + 1]["out"], np.float32) \
            + host_bias
    kernel._last_result = res
    return out


# revision 31
# speedup vs baseline: 1.1283x; 1.0484x over previous
"""Multi-head causal attention (B=4, T=2048, D=1024, H=16) on 8 trn2 NeuronCores.

Sharding: core c handles batch b = c//2 and head-group g = c%2 (8 heads each).
Each core computes Q/K/V projections for its 8 heads, causal attention, and a
row-shard of the output projection; the host sums the two partial outputs per
batch (the "all-reduce") and adds the (b_v @ w_o + b_o) bias term.

Device algebra notes:
  - b_k drops out of softmax entirely (adds a per-query constant to scores).
  - b_v contributes exactly (b_v @ w_o) to every output row -> folded into the
    host-side bias along with b_o.
  - Scores are computed transposed ([k, q] layout) so that softmax sums ride
    free on the AV matmul (ones-column appended to V) and the AV output comes
    out as AV^T, which feeds the w_o matmul with no extra transposes.
  - Softmax denominators use reciprocal_approx_fast (~51 ULP): the exact
    iterative-divide reciprocal is ~6 cycles/elem on DVE and was a hidden
    critical-path hog; bf16 AV noise dwarfs the approx error.
  - Exp is issued once per PAIR of k-tiles ([128, 2, 2, 512] from one PSUM
    region): the ACT engine is the s3 bottleneck and pays ~290ns fixed cost
    per instruction, so halving the instruction count matters. For diagonal
    tile pairs the odd member's [q0-128, q0) columns get exp'd garbage —
    never read (AV and the mask-mul slice per-subtile).
  - Output partials are stored bf16 (halves store traffic); host upcasts.

DMA rule: hardware DMA descriptors encode at most ONE semaphore wait, so every
DMA destination here is written exactly once (no pool-slot reuse for DMA
targets); partition broadcasts are done with PE outer products, not DMA.
"""

import math
from contextlib import ExitStack

import numpy as np

import concourse.bass as bass
import concourse.mybir as mybir
import concourse.tile as tile
from concourse import bacc
from concourse.bass_utils import run_bass_kernel_spmd


FP = mybir.dt.float32
BF = mybir.dt.bfloat16

D_MODEL = 1024
N_HEADS = 16
B_FULL, T_FULL = 4, 2048
DK = 64                    # head dim
HPC = 8                    # heads per core
DH = HPC * DK              # 512 head-dims per core
N_CORES = 8


def build_bass(seq_len=T_FULL, causal=True, repeat=1, stages='123F',
               score_pack=True, pipe=3, pair_feeds=3, s2_order0=True,
               warmup=16):
    """Build the per-core Bass kernel (SPMD; same NEFF on all 8 cores).

    repeat > 1 wraps the whole kernel in a hardware loop — used only for
    benchmarking (amortizes host dispatch to time the kernel itself).
    """
    T = seq_len
    NT = T // 128             # t-tiles
    NCH = T // 512            # 512-wide t/q chunks
    ND = D_MODEL // 128       # d_model tiles (8)
    NM = DH // 128            # head-pair tiles (4)

    nc = bacc.Bacc("TRN2", target_bir_lowering=False, debug=False)
    # inputs arrive pre-cast to bf16 AND pre-transposed (host does both):
    # xbf is x^T [D_MODEL, T], so the on-device x^T tile loads are plain
    # contiguous DMAs instead of xbar-transpose DMAs.
    xbf_d = nc.dram_tensor("xbf", [D_MODEL, T], BF, kind="ExternalInput")
    # weights arrive pre-arranged to the SBUF tile layout [p, j, c]
    # (host permutes), so each load is one contiguous-per-partition DMA
    wq_d = nc.dram_tensor("wq", [128, ND, DH], BF, kind="ExternalInput")
    wk_d = nc.dram_tensor("wk", [128, ND, DH], BF, kind="ExternalInput")
    wv_d = nc.dram_tensor("wv", [128, ND, DH], BF, kind="ExternalInput")
    wo_d = nc.dram_tensor("wo", [128, NM, D_MODEL], BF, kind="ExternalInput")
    bq_d = nc.dram_tensor("bq", [128, DH // 128], FP, kind="ExternalInput")
    out_d = nc.dram_tensor("out", [T, D_MODEL], BF, kind="ExternalOutput")

    with ExitStack() as ctx:
        tc = ctx.enter_context(tile.TileContext(nc))
        persist = ctx.enter_context(tc.tile_pool(name="persist", bufs=1))
        qt_pool = ctx.enter_context(tc.tile_pool(name="qt", bufs=3))
        xt_pool = ctx.enter_context(tc.tile_pool(name="xt", bufs=3))
        at_pool = ctx.enter_context(tc.tile_pool(name="atp", bufs=6))
        rec_pool = ctx.enter_context(tc.tile_pool(name="rec", bufs=2))
        avn_pool = ctx.enter_context(tc.tile_pool(name="avn", bufs=2))
        out_pool = ctx.enter_context(tc.tile_pool(name="outp", bufs=3))
        mm_ps = ctx.enter_context(tc.tile_pool(name="mmps", bufs=2, space="PSUM"))
        sc_ps = ctx.enter_context(tc.tile_pool(name="scps", bufs=2, space="PSUM"))
        av_ps = ctx.enter_context(tc.tile_pool(name="avps", bufs=1, space="PSUM"))

        def emit_kernel():
            # ---- weights + first x chunk first, so PE starts ASAP ----------
            def load_weight_bf(dram, n_tiles, cols, label):
                wsb = persist.tile([128, n_tiles, cols], BF, name=f"w_{label}",
                                   tag=f"w_{label}")
                nc.sync.dma_start(out=wsb, in_=dram.ap())
                return [wsb[:, j, :] for j in range(n_tiles)]

            def emit_s1(n):
                xT = [xt_pool.tile([128, 512], BF, name=f"xT{j}",
                                   tag=f"xT{j}") for j in range(ND)]
                for j in range(ND):
                    nc.sync.dma_start(out=xT[j],
                                      in_=xbf_d[j * 128:(j + 1) * 128,
                                                n * 512:(n + 1) * 512])
                return xT

            wk_bf = load_weight_bf(wk_d, ND, DH, "k")
            xT_cur = emit_s1(0)
            wq_bf = load_weight_bf(wq_d, ND, DH, "q")
            bq_sb = persist.tile([128, NM], FP, name="bq_sb", tag="bq_sb")
            nc.sync.dma_start(out=bq_sb, in_=bq_d[:, :])
            wv_bf = load_weight_bf(wv_d, ND, DH, "v")
            wo_bf = load_weight_bf(wo_d, NM, D_MODEL, "o")

            # ---- constants -------------------------------------------------
            ones_bf = persist.tile([1, DK], BF, name="ones_bf", tag="ones_bf")
            nc.gpsimd.memset(ones_bf, 1.0)

            # optional HAM warm-up: dummy matmuls run while the first DMAs
            # land, keeping the PE activity window busy so real matmuls
            # start un-throttled (PE clock gate needs ~3.4us of activity)
            if warmup:
                warm_sb = persist.tile([1, 512], BF, name="warm_sb",
                                       tag="warm_sb")
                nc.gpsimd.memset(warm_sb, 1.0)
                for w in range(warmup):
                    wps = mm_ps.tile([DK, 512], FP, name="warm", tag="mm")
                    nc.tensor.matmul(wps, ones_bf, warm_sb,
                                     start=True, stop=True)

            masks = []
            if causal:
                for j in range(4):
                    m = persist.tile([128, 512], BF, name=f"mask{j}", tag=f"mask{j}")
                    nc.gpsimd.memset(m, 1.0)
                    # keep where (q - k - 128*j) >= 0, else 0
                    nc.gpsimd.affine_select(
                        out=m, in_=m, compare_op=mybir.AluOpType.is_ge,
                        fill=0.0, base=-128 * j, pattern=[[1, 512]],
                        channel_multiplier=-1)
                    masks.append(m)

            def bcast_mid(ap, nmid):
                return bass.AP(tensor=ap.tensor, offset=ap.offset,
                               ap=[list(ap.ap[0]), [0, nmid], list(ap.ap[1])])

            # ---- persistent per-core tensors ------------------------------
            # KT[mt]: [128, T] bf16, rows = head-dim (pair mt: heads 2mt,2mt+1)
            KT = [persist.tile([128, T], BF, name=f"KT{mt}", tag=f"KT{mt}")
                  for mt in range(NM)]
            # V natural layout with ones column: [128 t, 8 heads, 64+1]
            V_sb = [persist.tile([128, HPC, DK + 1], BF, name=f"V{i}", tag=f"V{i}")
                    for i in range(NT)]
            for i in range(NT):
                nc.gpsimd.memset(V_sb[i][:, :, DK], 1.0)
            # AV^T, write-once (DMA target for the h1 partition shift)
            AVT_all = [[persist.tile([128, 512], BF, name=f"AVT{n}_{p}",
                                     tag=f"AVT{n}_{p}") for p in range(NM)]
                       for n in range(NCH)]

            # ---- main streamed loop over 512-wide chunks -------------------
            # S2(n+1) and F(n-1) matmul groups are fed into S3(n)'s group
            # boundaries: the per-engine queues are in-order, so S3's
            # exp-latency bubbles can only be filled by positionally
            # interleaving independent work into its instruction stream.

            def make_s2_groups(n, xT):
                csl = bass.ts(n, 512)
                QT = [qt_pool.tile([128, 512], BF, name=f"QT{mt}",
                                   tag=f"QT{mt}") for mt in range(NM)]
                groups = []

                def qgroup(mt):
                    msl = bass.ts(mt, 128)
                    ps = mm_ps.tile([128, 512], FP, name="psq", tag="mm")
                    for j in range(ND):
                        nc.tensor.matmul(ps, wq_bf[j][:, msl], xT[j],
                                         start=(j == 0), stop=(j == ND - 1))
                    nc.vector.tensor_scalar_add(QT[mt], ps, bq_sb[:, mt:mt + 1])

                def kgroup(mt):
                    msl = bass.ts(mt, 128)
                    ps = mm_ps.tile([128, 512], FP, name="psk", tag="mm")
                    for j in range(ND):
                        nc.tensor.matmul(ps, wk_bf[j][:, msl], xT[j],
                                         start=(j == 0), stop=(j == ND - 1))
                    # fold the 1/sqrt(dk) score scale into K^T so exp needs
                    # no scale and score magnitudes stay ~N(0,1)
                    nc.vector.tensor_scalar_mul(KT[mt][:, csl], ps,
                                                1.0 / math.sqrt(DK))

                def vgroup(il):
                    i = 4 * n + il
                    ps = mm_ps.tile([128, 512], FP, name="psv", tag="mm")
                    for j in range(ND):
                        nc.tensor.matmul(ps, xT[j][:, il * 128:(il + 1) * 128],
                                         wv_bf[j], start=(j == 0),
                                         stop=(j == ND - 1))
                    nc.vector.tensor_copy(
                        V_sb[i][:, :, 0:DK],
                        ps.rearrange("p (h d) -> p h d", h=HPC))

                if '2' in stages:
                    if n == 0 and s2_order0:
                        # chunk 0 runs un-interleaved at kernel start: order
                        # groups to match weight DMA arrival (wk, wq, wv)
                        for mt in range(NM):
                            groups.append(lambda mt=mt: kgroup(mt))
                        for mt in range(NM):
                            groups.append(lambda mt=mt: qgroup(mt))
                    else:
                        for mt in range(NM):
                            groups.append(lambda mt=mt: kgroup(mt))
                            groups.append(lambda mt=mt: qgroup(mt))
                    for il in range(4):
                        groups.append(lambda il=il: vgroup(il))
                return QT, groups

            def make_f_groups(n):
                AVT = AVT_all[n]
                groups = []

                osbs = {}

                def fgroup(il, cc):
                    i = 4 * n + il
                    isl = bass.ts(il, 128)
                    if il not in osbs:
                        osbs[il] = out_pool.tile([128, 2, 512], BF,
                                                 name="osb", tag="osb")
                    osb = osbs[il]
                    ps = mm_ps.tile([128, 512], FP, name="pso", tag="mm")
                    for dk in range(NM):
                        nc.tensor.matmul(
                            ps, AVT[dk][:, isl],
                            wo_bf[dk][:, cc * 512:(cc + 1) * 512],
                            start=(dk == 0), stop=(dk == NM - 1))
                    nc.vector.tensor_copy(osb[:, cc, :], ps)
                    nc.sync.dma_start(
                        out=out_d[i * 128:(i + 1) * 128,
                                  cc * 512:(cc + 1) * 512],
                        in_=osb[:, cc, :])

                if 'F' in stages:
                    for il in range(4):
                        for cc in range(2):
                            groups.append(lambda il=il, cc=cc: fgroup(il, cc))
                return groups

            def emit_s3(n, QT, feed):
                AVT = AVT_all[n]
                nkt = 4 * n + 4 if causal else NT
                PIPE = pipe
                pending_norm = [None]

                def emit_norm(p, av):
                    den_bf = rec_pool.tile([1, 2, 512], BF, name="den_bf",
                                           tag="den_bf")
                    nc.vector.tensor_copy(den_bf, av[DK:DK + 1, :, :])
                    # broadcast both heads' denominators with col-packed
                    # (concurrent) K=1 matmuls into disjoint psum partitions
                    rb = mm_ps.tile([128, 512], FP, name="rb", tag="mm")
                    for hh in range(2):
                        nc.tensor.matmul(rb[hh * 64:(hh + 1) * 64, :],
                                         ones_bf, den_bf[:, hh, :],
                                         start=True, stop=True,
                                         tile_position=(0, hh * 64))
                    # ~6 cpe exact reciprocal is a hidden DVE hog; the
                    # ~51-ULP approx dwarfs bf16 AV noise
                    rb_sb = rec_pool.tile([128, 512], FP, name="rb_sb",
                                          tag="rb_sb")
                    nc.vector.reciprocal_approx_fast(rb_sb, rb)
                    nc.vector.tensor_mul(AVT[p][0:64, :], av[0:DK, 0, :],
                                         rb_sb[0:64, :])
                    avn1 = avn_pool.tile([64, 512], BF, name="avn1",
                                         tag="avn1")
                    nc.vector.tensor_mul(avn1, av[0:DK, 1, :],
                                         rb_sb[64:128, :])
                    nc.sync.dma_start(out=AVT[p][64:128, :], in_=avn1)

                npairs = NM if '3' in stages else 0
                for p in range(npairs):
                    av = av_ps.tile([DK + 1, 2, 512], FP, name="av", tag="av")

                    def emit_av(kt, at, q0):
                        for hh in range(2):
                            nc.tensor.matmul(
                                av[:, hh, q0:512],
                                V_sb[kt][:, 2 * p + hh, :],
                                at[:, hh, q0:512],
                                start=(kt == 0), stop=(kt == nkt - 1),
                                skip_group_check=True)

                    pend = []
                    for kt in range(nkt):
                        ksl = bass.ts(kt, 128)
                        # columns < j*128 of a diagonal tile are fully
                        # masked: skip in scores matmul / exp / mask / AV
                        j = kt - 4 * n if (causal and kt >= 4 * n) else 0
                        q0 = j * 128
                        ps_s = sc_ps.tile([128, 2, 512], FP, name="ps_s",
                                          tag="sc")
                        at = at_pool.tile([128, 2, 512], BF, name="at",
                                          tag="at")
                        for hh in range(2):
                            nc.tensor.matmul(
                                ps_s[:, hh, q0:512],
                                KT[p][hh * 64:(hh + 1) * 64, ksl],
                                QT[p][hh * 64:(hh + 1) * 64, q0:512],
                                start=True, stop=True,
                                tile_position=((hh * 64, 0) if score_pack
                                               else (0, 0)))
                        nc.scalar.activation(at[:, :, q0:512],
                                             ps_s[:, :, q0:512],
                                             mybir.ActivationFunctionType.Exp)
                        if causal and kt >= 4 * n:
                            # only the 128-wide diagonal block is partially
                            # masked; columns >= q0+128 are fully unmasked
                            q1 = q0 + 128
                            nc.vector.tensor_mul(
                                at[:, :, q0:q1], at[:, :, q0:q1],
                                bcast_mid(masks[j][:, q0:q1], 2))
                        if kt == 0 and pending_norm[0] is not None:
                            pending_norm[0]()
                            pending_norm[0] = None
                        pend.append((kt, at, q0))
                        if len(pend) > PIPE:
                            emit_av(*pend.pop(0))
                    for item in pend:
                        emit_av(*item)
                    pending_norm[0] = (lambda p=p, av=av:
                                       emit_norm(p, av))
                    # fill the pair-transition bubble with independent work
                    for _ in range(pair_feeds):
                        if feed:
                            feed.pop(0)()
                if pending_norm[0] is not None:
                    pending_norm[0]()
                    pending_norm[0] = None
                while feed:
                    feed.pop(0)()

            QT_cur, s2g = make_s2_groups(0, xT_cur)
            for g in s2g:
                g()
            f_prev = []
            for n in range(NCH):
                feed = []
                if n + 1 < NCH:
                    xT_next = emit_s1(n + 1)
                    QT_next, s2g_next = make_s2_groups(n + 1, xT_next)
                    feed += s2g_next
                else:
                    QT_next = None
                feed += f_prev
                if '3' in stages:
                    emit_s3(n, QT_cur, feed)
                else:
                    for g in feed:
                        g()
                f_prev = make_f_groups(n)
                QT_cur = QT_next
            for g in f_prev:
                g()

        if repeat > 1:
            with tc.For_i(0, repeat, 1):
                emit_kernel()
        else:
            emit_kernel()

    nc.compile()
    return nc


_NC_CACHE = {}


def _get_nc(seq_len, causal):
    key = (seq_len, causal)
    if key not in _NC_CACHE:
        _NC_CACHE[key] = build_bass(seq_len, causal)
    return _NC_CACHE[key]


def make_in_maps(x, w_q, b_q, w_k, w_v, w_o):
    """Per-core input dicts for the 8 cores (weights/x pre-cast to bf16)."""
    import ml_dtypes
    bf = ml_dtypes.bfloat16
    # pre-transpose per batch: device expects x^T [D_MODEL, T]
    x_bf = np.ascontiguousarray(np.asarray(x).transpose(0, 2, 1)).astype(bf)
    wq_bf = w_q.astype(bf)
    wk_bf = w_k.astype(bf)
    wv_bf = w_v.astype(bf)
    wo_bf = w_o.astype(bf)
    def sbuf_layout(w):  # [rows, cols] -> [128, rows//128, cols], p-major
        r, c = w.shape
        return np.ascontiguousarray(
            w.reshape(r // 128, 128, c).transpose(1, 0, 2))

    in_maps = []
    for c in range(N_CORES):
        b, g = divmod(c, 2)
        sl = slice(g * DH, (g + 1) * DH)
        in_maps.append({
            "xbf": x_bf[b],
            "wq": sbuf_layout(wq_bf[:, sl]),
            "wk": sbuf_layout(wk_bf[:, sl]),
            "wv": sbuf_layout(wv_bf[:, sl]),
            "wo": sbuf_layout(np.ascontiguousarray(wo_bf[sl, :])),
            "bq": np.ascontiguousarray(
                b_q[sl].reshape(DH // 128, 128).T.astype(np.float32)),
        })
    return in_maps


def kernel(x, mask, w_q, b_q, w_k, b_k, w_v, b_v, w_o, b_o, _trace=False):
    x = np.asarray(x, dtype=np.float32)
    mask_np = np.asarray(mask).reshape(mask.shape[-2], mask.shape[-1])
    w_q, b_q = np.asarray(w_q, np.float32), np.asarray(b_q, np.float32)
    w_k = np.asarray(w_k, np.float32)
    w_v, b_v = np.asarray(w_v, np.float32), np.asarray(b_v, np.float32)
    w_o, b_o = np.asarray(w_o, np.float32), np.asarray(b_o, np.float32)

    T = x.shape[1]
    tril = np.tril(np.ones((T, T), dtype=mask_np.dtype))
    if np.array_equal(mask_np, tril):
        causal = True
    elif np.all(mask_np != 0):
        causal = False
    else:
        raise NotImplementedError("only causal or all-ones masks supported")

    nc = _get_nc(T, causal)
    in_maps = make_in_maps(x, w_q, b_q, w_k, w_v, w_o)
    res = run_bass_kernel_spmd(nc, in_maps, core_ids=list(range(N_CORES)),
                               trace=_trace)

    host_bias = (b_v @ w_o + b_o).astype(np.float32)
    out = np.empty((x.shape[0], T, D_MODEL), dtype=np.float32)
    for b in range(x.shape[0]):
        out[b] = np.asarray(res.results[2 * b]["out"], np.float32) \
            + np.asarray(res.results[2 * b + 1]["out"], np.float32) \
            + host_bias
    kernel._last_result = res
    return out
